# revision 5
# baseline (speedup 1.0000x reference)
"""Trainium2 Bass kernel for nn_PhaseAttentionLM.

Model: x = tok_emb[idx] + pos_emb; 2x low-rank linear attention (k=64,
causal cumsum) with residual+LN; final LN; MLP 1024->4096 (erf gelu)
-> 32000 logits.

Sharding (8 cores): 2-way data parallel over batch x 4-way tensor
parallel over vocab. Each core runs the full front-end for its batch row
and computes a [2048, 8000] logits slice (padded to 8192). No
collectives; host concatenates.

All heavy GEMMs run as float32r (TF32-like, 1 cyc/row at free-dim>=256,
~1e-4 rounding). Attention state is a 64x65 PSUM accumulator passed
across 16 chunks of 128 positions; within-chunk causality is a masked
QK^T matmul.
"""
import sys

if "/opt/trn_rl_repo" not in sys.path:
    sys.path.insert(0, "/opt/trn_rl_repo")

import numpy as np

import concourse.bacc as bacc
import concourse.bass as bass
import concourse.mybir as mybir
import concourse.tile as tile
from concourse.bass import IndirectOffsetOnAxis, ts
from concourse.masks import make_identity, make_upper_triangular
from concourse.bass_utils import run_bass_kernel_spmd

P = 128
D = 1024           # model dim; 8 k-tiles
KTILES = D // P    # 8
L = 2048           # sequence per core (one batch row); 16 pos tiles
NT = L // P        # 16 pos tiles
NPB = L // 512     # 4 pos blocks of 512
KD = 64            # attention key dim
NH = 2             # layers
HID = 4096         # 32 hid tiles
JT = HID // P      # 32
VOCAB = 32000
VPC = 8192         # padded vocab per core (4-way split of 32000 -> 8000 -> pad)
VB = 256           # vocab block (free dim of W2 matmuls)
NVB = VPC // VB    # 32
HALF = 1024        # pos half for the MLP phase
EPS = 1e-5
DEN_EPS = 1e-6

F32 = mybir.dt.float32
F32R = mybir.dt.float32r
I32 = mybir.dt.int32
AF = mybir.ActivationFunctionType
ALU = mybir.AluOpType

_cached = None


def _layernorm_block(nc, pools, xT, pb, g_sb, b_sb, onesc, onesr):
    """LN over feature dim for pos block pb (512 cols), in xT layout
    [128, KTILES, L] (partition=feat). Stats via ones-matmul reductions,
    broadcast via K=1 matmul, normalize on DVE, in-place into xT."""
    lnp, sqp, xnp, bcp = pools
    pbs = ts(pb, 512)
    mp = lnp.tile([1, 512], F32, name="ln_mean", space="PSUM")
    sp = lnp.tile([1, 512], F32, name="ln_sumsq", space="PSUM")
    for k in range(KTILES):
        sq = sqp.tile([P, 512], F32R, name="ln_sq")
        nc.vector.tensor_mul(sq[:], xT[:, k, pbs], xT[:, k, pbs])
        nc.tensor.matmul(mp[:], lhsT=onesc[:], rhs=xT[:, k, pbs],
                         start=(k == 0), stop=(k == KTILES - 1))
        nc.tensor.matmul(sp[:], lhsT=onesc[:], rhs=sq[:],
                         start=(k == 0), stop=(k == KTILES - 1))
    m_sb = sqp.tile([1, 512], F32, name="ln_m")
    v_sb = sqp.tile([1, 512], F32, name="ln_v")
    msq = sqp.tile([1, 512], F32, name="ln_msq")
    rv = sqp.tile([1, 512], F32, name="ln_rv")
    a_sb = sqp.tile([1, 512], F32R, name="ln_a")
    c_f = sqp.tile([1, 512], F32, name="ln_cf")
    c_sb = sqp.tile([1, 512], F32R, name="ln_c")
    nc.vector.tensor_scalar_mul(m_sb[:], mp[:], 1.0 / D)
    nc.vector.tensor_scalar_mul(v_sb[:], sp[:], 1.0 / D)
    nc.vector.tensor_mul(msq[:], m_sb[:], m_sb[:])
    nc.vector.tensor_sub(v_sb[:], v_sb[:], msq[:])
    nc.vector.tensor_scalar_add(v_sb[:], v_sb[:], EPS)
    nc.vector.reciprocal(rv[:], v_sb[:])
    nc.scalar.activation(a_sb[:], rv[:], AF.Sqrt)          # rstd
    nc.vector.tensor_mul(c_f[:], m_sb[:], a_sb[:])
    nc.vector.tensor_scalar_mul(c_sb[:], c_f[:], -1.0)     # -mean*rstd
    ab = bcp.tile([P, 512], F32, name="ln_ab", space="PSUM")
    cb = bcp.tile([P, 512], F32, name="ln_cb", space="PSUM")
    nc.tensor.matmul(ab[:], lhsT=onesr[:], rhs=a_sb[:], start=True, stop=True)
    nc.tensor.matmul(cb[:], lhsT=onesr[:], rhs=c_sb[:], start=True, stop=True)
    for k in range(KTILES):
        xn = xnp.tile([P, 512], F32, name="ln_xn")
        nc.vector.tensor_mul(xn[:], xT[:, k, pbs], ab[:])
        nc.vector.tensor_add(xn[:], xn[:], cb[:])
        nc.vector.tensor_scalar(
            out=xT[:, k, pbs], in0=xn[:],
            scalar1=g_sb[:, k:k + 1], scalar2=b_sb[:, k:k + 1],
            op0=ALU.mult, op1=ALU.add,
        )


def build():
    nc = bacc.Bacc("TRN2", target_bir_lowering=False, debug=False)

    idx_d = nc.dram_tensor("idx", [P, NT], I32, kind="ExternalInput")
    tok_d = nc.dram_tensor("tok_emb", [VOCAB, D], F32, kind="ExternalInput")
    pos_d = nc.dram_tensor("pos_emb", [L, D], F32, kind="ExternalInput")
    wq_d = nc.dram_tensor("wq", [NH, P, KTILES, KD], F32R, kind="ExternalInput")
    wk_d = nc.dram_tensor("wk", [NH, P, KTILES, KD], F32R, kind="ExternalInput")
    wv_d = nc.dram_tensor("wv", [NH, P, KTILES, KD], F32R, kind="ExternalInput")
    wo_d = nc.dram_tensor("wo", [NH, KD, KTILES, P], F32R, kind="ExternalInput")
    ng_d = nc.dram_tensor("ng", [NH, P, KTILES], F32, kind="ExternalInput")
    nb_d = nc.dram_tensor("nb", [NH, P, KTILES], F32, kind="ExternalInput")
    og_d = nc.dram_tensor("og", [P, KTILES], F32, kind="ExternalInput")
    ob_d = nc.dram_tensor("ob", [P, KTILES], F32, kind="ExternalInput")
    w1_d = nc.dram_tensor("w1", [P, JT, KTILES, P], F32R, kind="ExternalInput")
    b1_d = nc.dram_tensor("b1", [P, JT], F32, kind="ExternalInput")
    w2_d = nc.dram_tensor("w2", [P, NVB, JT, VB], F32R, kind="ExternalInput")
    b2_d = nc.dram_tensor("b2", [NVB, VB], F32R, kind="ExternalInput")
    out_d = nc.dram_tensor("logits", [L, VPC], F32, kind="ExternalOutput")
    xtn_d = nc.dram_tensor("xtn_scratch", [P, KTILES, L], F32R)

    with tile.TileContext(nc) as tc:
        with tc.tile_pool(name="persist", bufs=1) as pp:
            ident = pp.tile([P, P], F32)
            make_identity(nc, ident[:])
            mask = pp.tile([P, P], F32)           # keep k<=q (upper incl diag)
            make_upper_triangular(nc, mask[:], val=1.0, diag=True)
            onesc_f = pp.tile([P, 1], F32)
            onesr_f = pp.tile([1, P], F32)
            nc.vector.memset(onesc_f[:], 1.0)
            nc.vector.memset(onesr_f[:], 1.0)
            onesc = pp.tile([P, 1], F32R)
            onesr = pp.tile([1, P], F32R)
            nc.vector.tensor_copy(onesc[:], onesc_f[:])
            nc.vector.tensor_copy(onesr[:], onesr_f[:])
            idx_sb = pp.tile([P, NT], I32)
            nc.sync.dma_start(idx_sb[:], idx_d[:])
            og_sb = pp.tile([P, KTILES], F32)
            ob_sb = pp.tile([P, KTILES], F32)
            nc.sync.dma_start(og_sb[:], og_d[:])
            nc.sync.dma_start(ob_sb[:], ob_d[:])
            xT = pp.tile([P, KTILES, L], F32R)    # x^T, resident

            # ---------------- embed: gather + pos add + transpose ----------
            with tc.tile_pool(name="emb", bufs=3) as ep, \
                 tc.tile_pool(name="embp", bufs=4, space="PSUM") as epp:
                for t in range(NT):
                    xt = ep.tile([P, D], F32, name="xt")
                    nc.gpsimd.indirect_dma_start(
                        out=xt[:], out_offset=None, in_=tok_d[:],
                        in_offset=IndirectOffsetOnAxis(ap=idx_sb[:, t:t + 1], axis=0),
                    )
                    pe = ep.tile([P, D], F32, name="pe")
                    nc.sync.dma_start(pe[:], pos_d[ts(t, P), :])
                    nc.vector.tensor_add(xt[:], xt[:], pe[:])
                    for k in range(KTILES):
                        tp = epp.tile([P, P], F32, name="tp")
                        nc.tensor.transpose(tp[:], xt[:, ts(k, P)], ident[:])
                        nc.vector.tensor_copy(xT[:, k, ts(t, P)], tp[:])

            # ---------------- attention layers ------------------------------
            for layer in range(NH):
                with tc.tile_pool(name=f"lyr{layer}", bufs=1) as lp:
                    wq_sb = lp.tile([P, KTILES, KD], F32R)
                    wk_sb = lp.tile([P, KTILES, KD], F32R)
                    wv_sb = lp.tile([P, KTILES, KD], F32R)
                    wo_sb = lp.tile([KD, KTILES, P], F32R)
                    ng_sb = lp.tile([P, KTILES], F32)
                    nb_sb = lp.tile([P, KTILES], F32)
                    nc.sync.dma_start(wq_sb[:], wq_d[layer])
                    nc.sync.dma_start(wk_sb[:], wk_d[layer])
                    nc.sync.dma_start(wv_sb[:], wv_d[layer])
                    nc.sync.dma_start(wo_sb[:], wo_d[layer])
                    nc.sync.dma_start(ng_sb[:], ng_d[layer])
                    nc.sync.dma_start(nb_sb[:], nb_d[layer])
                    QT = lp.tile([KD, L], F32R)
                    KT = lp.tile([KD, L], F32R)
                    V_aug = lp.tile([P, NT, KD + 2], F32R)
                    K_pf = lp.tile([P, NT, KD], F32R)
                    nT = lp.tile([KD, L], F32R)
                    S_sb = lp.tile([KD, KD + 2], F32R)
                    zs = lp.tile([KD, KD + 2], F32)
                    zc_f = lp.tile([P, 1], F32)
                    nc.vector.memset(zc_f[:], 0.0)
                    nc.vector.memset(zs[:], 0.0)
                    nc.vector.tensor_copy(S_sb[:], zs[:])

                    # Q^T, K^T projections: [64, 512] psum per pos block
                    with tc.tile_pool(name="projp", bufs=3, space="PSUM") as qp_pool, \
                         tc.tile_pool(name="projs", bufs=3) as qs_pool:
                        for pb in range(NPB):
                            pbs = ts(pb, 512)
                            for w_sb, OUT in ((wq_sb, QT), (wk_sb, KT)):
                                qp = qp_pool.tile([KD, 512], F32, name="qp",
                                                  space="PSUM")
                                for k in range(KTILES):
                                    nc.tensor.matmul(
                                        qp[:], lhsT=w_sb[:, k, :],
                                        rhs=xT[:, k, pbs],
                                        start=(k == 0), stop=(k == KTILES - 1),
                                    )
                                # elu(z)+1 = exp(min(z,0)) + max(z,0)
                                t1 = qs_pool.tile([KD, 512], F32, name="t1")
                                t2 = qs_pool.tile([KD, 512], F32, name="t2")
                                nc.vector.tensor_scalar_min(t1[:], qp[:], 0.0)
                                nc.scalar.activation(t1[:], t1[:], AF.Exp)
                                nc.vector.tensor_scalar_max(t2[:], qp[:], 0.0)
                                nc.vector.tensor_add(OUT[:, pbs], t1[:], t2[:])

                    # V = x @ Wv in [pos, kd] layout; K_pf = transpose of the
                    # activated KT (S update must see elu(K)+1)
                    with tc.tile_pool(name="vkp", bufs=4, space="PSUM") as vk_pool:
                        for t in range(NT):
                            vp = vk_pool.tile([P, KD], F32, name="vp", space="PSUM")
                            for k in range(KTILES):
                                nc.tensor.matmul(vp[:], lhsT=xT[:, k, ts(t, P)],
                                                 rhs=wv_sb[:, k, :],
                                                 start=(k == 0), stop=(k == KTILES - 1))
                            nc.vector.tensor_copy(V_aug[:, t, 0:KD], vp[:])
                            nc.vector.tensor_copy(V_aug[:, t, KD:KD + 1], onesc_f[:])
                            nc.vector.tensor_copy(V_aug[:, t, KD + 1:KD + 2],
                                                  zc_f[:])
                            ktr = vk_pool.tile([P, KD], F32, name="ktr", space="PSUM")
                            nc.tensor.transpose(
                                ktr[:], KT[:, ts(t, P)].bitcast(F32),
                                ident[:KD, :KD])
                            nc.vector.tensor_copy(K_pf[:, t, :], ktr[:])

                    # causal chunk scan
                    with tc.tile_pool(name="scanp", bufs=2, space="PSUM") as sc_pool, \
                         tc.tile_pool(name="sps", bufs=1, space="PSUM") as s_pool, \
                         tc.tile_pool(name="scans", bufs=3) as ss_pool:
                        S_ps = s_pool.tile([KD, KD + 2], F32, name="s_ps",
                                           space="PSUM")
                        for t in range(NT):
                            tts = ts(t, P)
                            at = sc_pool.tile([P, P], F32, name="at", space="PSUM")
                            nc.tensor.matmul(at[:], lhsT=KT[:, tts], rhs=QT[:, tts],
                                             start=True, stop=True)
                            atm = ss_pool.tile([P, P], F32R, name="atm")
                            nc.vector.tensor_tensor(
                                out=atm[:], in0=at[:], in1=mask[:], op=ALU.mult)
                            np_ = sc_pool.tile([P, KD + 2], F32, name="np",
                                               space="PSUM")
                            nc.tensor.matmul(np_[:], lhsT=atm[:], rhs=V_aug[:, t, :],
                                             start=True, stop=False)
                            nc.tensor.matmul(np_[:], lhsT=QT[:, tts], rhs=S_sb[:],
                                             start=False, stop=True)
                            nc.tensor.matmul(S_ps[:], lhsT=K_pf[:, t, :],
                                             rhs=V_aug[:, t, :],
                                             start=(t == 0), stop=(t == NT - 1))
                            nc.vector.tensor_copy(S_sb[:], S_ps[:])
                            rd = ss_pool.tile([P, 1], F32, name="rd")
                            rec = ss_pool.tile([P, 1], F32, name="rec")
                            n_sb = ss_pool.tile([P, KD], F32, name="n_sb")
                            nc.vector.tensor_scalar_add(rd[:], np_[:, KD:KD + 1],
                                                        DEN_EPS)
                            nc.vector.reciprocal(rec[:], rd[:])
                            nc.vector.tensor_scalar_mul(n_sb[:], np_[:, 0:KD], rec[:])
                            tr = sc_pool.tile([KD, P], F32, name="tr", space="PSUM")
                            nc.tensor.transpose(tr[:], n_sb[:], ident[:])
                            nc.vector.tensor_copy(nT[:, tts], tr[:])

                    # Wo apply + residual + LN, per pos block
                    with tc.tile_pool(name="wop", bufs=2, space="PSUM") as wo_pool, \
                         tc.tile_pool(name="lnp", bufs=1, space="PSUM") as ln_pool, \
                         tc.tile_pool(name="lnsq", bufs=2) as sq_pool, \
                         tc.tile_pool(name="lnxn", bufs=2) as xn_pool, \
                         tc.tile_pool(name="lnbc", bufs=1, space="PSUM") as bc_pool:
                        for pb in range(NPB):
                            pbs = ts(pb, 512)
                            for k in range(KTILES):
                                yp = wo_pool.tile([P, 512], F32, name="yp",
                                                  space="PSUM")
                                nc.tensor.matmul(yp[:], lhsT=wo_sb[:, k, :],
                                                 rhs=nT[:, pbs],
                                                 start=True, stop=True)
                                nc.vector.tensor_add(xT[:, k, pbs], xT[:, k, pbs],
                                                     yp[:])
                            _layernorm_block(
                                nc, (ln_pool, sq_pool, xn_pool, bc_pool),
                                xT, pb, ng_sb, nb_sb, onesc, onesr)

            # ---------------- final LN + spill xT ---------------------------
            with tc.tile_pool(name="flnp", bufs=1, space="PSUM") as ln_pool, \
                 tc.tile_pool(name="flnsq", bufs=2) as sq_pool, \
                 tc.tile_pool(name="flnxn", bufs=2) as xn_pool, \
                 tc.tile_pool(name="flnbc", bufs=1, space="PSUM") as bc_pool:
                for pb in range(NPB):
                    _layernorm_block(
                        nc, (ln_pool, sq_pool, xn_pool, bc_pool),
                        xT, pb, og_sb, ob_sb, onesc, onesr)
            for k in range(KTILES):
                nc.sync.dma_start(xtn_d[:, k, :], xT[:, k, :])

        # ---------------- MLP ----------------------------------------------
        with tc.tile_pool(name="mlp", bufs=1) as mp_pool:
            b1_sb = mp_pool.tile([P, JT], F32)
            nc.sync.dma_start(b1_sb[:], b1_d[:])
            onesr2_f = mp_pool.tile([1, P], F32)
            nc.vector.memset(onesr2_f[:], 1.0)
            onesr2 = mp_pool.tile([1, P], F32R)
            nc.vector.tensor_copy(onesr2[:], onesr2_f[:])
            hT = mp_pool.tile([P, JT, HALF], F32R)
            for half in range(2):
                hoff = half * HALF
                # ---- W1 GEMM + gelu -> hT ----
                with tc.tile_pool(name="w1x", bufs=2) as xp, \
                     tc.tile_pool(name="w1s", bufs=3) as w1s, \
                     tc.tile_pool(name="w1p", bufs=3, space="PSUM") as hp1:
                    xhbs = []
                    for hb in range(2):
                        xhb = xp.tile([P, KTILES, 512], F32R, name="xhb")
                        nc.sync.dma_start(
                            xhb[:], xtn_d[:, :, hoff + hb * 512:hoff + (hb + 1) * 512])
                        xhbs.append(xhb)
                    for j in range(JT):
                        w1t = w1s.tile([P, KTILES, P], F32R, name="w1t")
                        nc.sync.dma_start(w1t[:], w1_d[:, j, :, :])
                        for hb in range(2):
                            ps = hp1.tile([P, 512], F32, name="hps", space="PSUM")
                            for k in range(KTILES):
                                nc.tensor.matmul(ps[:], lhsT=w1t[:, k, :],
                                                 rhs=xhbs[hb][:, k, :],
                                                 start=(k == 0),
                                                 stop=(k == KTILES - 1))
                            nc.scalar.activation(
                                hT[:, j, ts(hb, 512)], ps[:], AF.Gelu,
                                bias=b1_sb[:, j:j + 1], scale=1.0)
                # ---- W2 GEMM -> logits ----
                with tc.tile_pool(name="w2s", bufs=2) as w2p, \
                     tc.tile_pool(name="w2lg", bufs=3) as lgp, \
                     tc.tile_pool(name="w2b", bufs=2) as b2p, \
                     tc.tile_pool(name="w2ps", bufs=4, space="PSUM") as lp2, \
                     tc.tile_pool(name="w2bp", bufs=1, space="PSUM") as bp2:
                    for vb in range(NVB):
                        w2t = w2p.tile([P, JT, VB], F32R, name="w2t")
                        nc.sync.dma_start(w2t[:], w2_d[:, vb, :, :])
                        b2v = b2p.tile([1, VB], F32R, name="b2v")
                        nc.sync.dma_start(b2v[:], b2_d[vb:vb + 1, :])
                        b2ps = bp2.tile([P, VB], F32, name="b2ps", space="PSUM")
                        nc.tensor.matmul(b2ps[:], lhsT=onesr2[:], rhs=b2v[:],
                                         start=True, stop=True)
                        b2b = b2p.tile([P, VB], F32, name="b2b")
                        nc.vector.tensor_copy(b2b[:], b2ps[:])
                        for pt in range(8):
                            ps = lp2.tile([P, VB], F32, name="lps", space="PSUM")
                            for k in range(JT):
                                nc.tensor.matmul(ps[:], lhsT=hT[:, k, ts(pt, P)],
                                                 rhs=w2t[:, k, :],
                                                 start=(k == 0), stop=(k == JT - 1))
                            lg = lgp.tile([P, VB], F32, name="lg")
                            nc.vector.tensor_add(lg[:], ps[:], b2b[:])
                            nc.sync.dma_start(
                                out_d[hoff + pt * P:hoff + (pt + 1) * P,
                                      ts(vb, VB)], lg[:])
    nc.compile()
    return nc


def _prep_core_inputs(inputs, batch, vslice):
    f32 = np.float32
    idx = np.asarray(inputs["input_indices"])[batch].astype(np.int32)
    idxr = np.ascontiguousarray(idx.reshape(NT, P).T)
    tok = np.ascontiguousarray(np.asarray(inputs["token_embed"], dtype=f32))
    pos = np.ascontiguousarray(np.asarray(inputs["pos_embed"], dtype=f32)[:L])
    wq = np.asarray(inputs["Wq"], dtype=f32).reshape(NH, KTILES, P, KD)
    wk = np.asarray(inputs["Wk"], dtype=f32).reshape(NH, KTILES, P, KD)
    wv = np.asarray(inputs["Wv"], dtype=f32).reshape(NH, KTILES, P, KD)
    wqr = np.ascontiguousarray(wq.transpose(0, 2, 1, 3))
    wkr = np.ascontiguousarray(wk.transpose(0, 2, 1, 3))
    wvr = np.ascontiguousarray(wv.transpose(0, 2, 1, 3))
    wo = np.ascontiguousarray(
        np.asarray(inputs["Wo"], dtype=f32).reshape(NH, KD, KTILES, P))
    ng = np.ascontiguousarray(
        np.asarray(inputs["norm_g"], dtype=f32).reshape(NH, KTILES, P)
        .transpose(0, 2, 1))
    nb = np.ascontiguousarray(
        np.asarray(inputs["norm_b"], dtype=f32).reshape(NH, KTILES, P)
        .transpose(0, 2, 1))
    og = np.ascontiguousarray(
        np.asarray(inputs["out_norm_g"], dtype=f32).reshape(KTILES, P).T)
    ob = np.ascontiguousarray(
        np.asarray(inputs["out_norm_b"], dtype=f32).reshape(KTILES, P).T)
    w1 = np.asarray(inputs["W1"], dtype=f32).reshape(KTILES, P, JT, P)
    w1r = np.ascontiguousarray(w1.transpose(1, 2, 0, 3))
    b1r = np.ascontiguousarray(
        np.asarray(inputs["b1"], dtype=f32).reshape(JT, P).T)
    w2 = np.asarray(inputs["W2"], dtype=f32)
    vs = VOCAB // 4
    w2p = np.zeros((HID, VPC), dtype=f32)
    w2p[:, :vs] = w2[:, vslice * vs:(vslice + 1) * vs]
    w2r = np.ascontiguousarray(
        w2p.reshape(JT, P, NVB, VB).transpose(1, 2, 0, 3))
    b2 = np.asarray(inputs["b2"], dtype=f32)
    b2p = np.zeros((VPC,), dtype=f32)
    b2p[:vs] = b2[vslice * vs:(vslice + 1) * vs]
    b2r = b2p.reshape(NVB, VB)
    return {
        "idx": idxr, "tok_emb": tok, "pos_emb": pos,
        "wq": wqr, "wk": wkr, "wv": wvr, "wo": wo,
        "ng": ng, "nb": nb, "og": og, "ob": ob,
        "w1": w1r, "b1": b1r, "w2": w2r, "b2": b2r,
    }


def kernel(**inputs) -> np.ndarray:
    global _cached
    if _cached is None:
        _cached = build()
    nc = _cached
    in_maps = [_prep_core_inputs(inputs, c // 4, c % 4) for c in range(8)]
    r = run_bass_kernel_spmd(nc, in_maps, core_ids=list(range(8)))
    vs = VOCAB // 4
    B = np.asarray(inputs["input_indices"]).shape[0]
    out = np.empty((B, L, VOCAB), dtype=np.float32)
    for c in range(8):
        b, v = c // 4, c % 4
        out[b, :, v * vs:(v + 1) * vs] = r.results[c]["logits"][:, :vs]
    return out


# revision 8
# speedup vs baseline: 1.0049x; 1.0049x over previous
"""Trainium2 Bass kernel for nn_PhaseAttentionLM.

Model: x = tok_emb[idx] + pos_emb; 2x low-rank linear attention (k=64,
causal cumsum) with residual+LN; final LN; MLP 1024->4096 (erf gelu)
-> 32000 logits.

Sharding (8 cores): 2-way data parallel over batch x 4-way tensor
parallel over vocab. Each core runs the full front-end for its batch row
and computes a [2048, 8000] logits slice (padded to 8192). No
collectives; host concatenates.

All heavy GEMMs run as float32r (TF32-like, 1 cyc/row at free-dim>=256,
~1e-4 rounding). Attention state is a 64x65 PSUM accumulator passed
across 16 chunks of 128 positions; within-chunk causality is a masked
QK^T matmul.
"""
import sys

if "/opt/trn_rl_repo" not in sys.path:
    sys.path.insert(0, "/opt/trn_rl_repo")

import numpy as np

import concourse.bacc as bacc
import concourse.bass as bass
import concourse.mybir as mybir
import concourse.tile as tile
from concourse.bass import IndirectOffsetOnAxis, ts
from concourse.masks import make_identity, make_upper_triangular
from concourse.bass_utils import run_bass_kernel_spmd

P = 128
D = 1024           # model dim; 8 k-tiles
KTILES = D // P    # 8
L = 2048           # sequence per core (one batch row); 16 pos tiles
NT = L // P        # 16 pos tiles
NPB = L // 512     # 4 pos blocks of 512
KD = 64            # attention key dim
NH = 2             # layers
HID = 4096         # 32 hid tiles
JT = HID // P      # 32
VOCAB = 32000
VPC = 8192         # padded vocab per core (4-way split of 32000 -> 8000 -> pad)
VB = 256           # vocab block (free dim of W2 matmuls)
NVB = VPC // VB    # 32
HALF = 1024        # pos half for the MLP phase
EPS = 1e-5
DEN_EPS = 1e-6

F32 = mybir.dt.float32
F32R = mybir.dt.float32r
I32 = mybir.dt.int32
AF = mybir.ActivationFunctionType
ALU = mybir.AluOpType

_cached = None


def _layernorm_block(nc, pools, xT, pb, g_sb, b_sb, onesc, onesr):
    """LN over feature dim for pos block pb (512 cols), in xT layout
    [128, KTILES, L] (partition=feat). Stats via ones-matmul reductions,
    broadcast via K=1 matmul, normalize on DVE, in-place into xT."""
    lnp, sqp, xnp, bcp = pools
    pbs = ts(pb, 512)
    mp = lnp.tile([1, 512], F32, name="ln_mean", space="PSUM")
    sp = lnp.tile([1, 512], F32, name="ln_sumsq", space="PSUM")
    for k in range(KTILES):
        sq = sqp.tile([P, 512], F32R, name="ln_sq")
        nc.scalar.activation(sq[:], xT[:, k, pbs], AF.Square)
        nc.tensor.matmul(mp[:], lhsT=onesc[:], rhs=xT[:, k, pbs],
                         start=(k == 0), stop=(k == KTILES - 1))
        nc.tensor.matmul(sp[:], lhsT=onesc[:], rhs=sq[:],
                         start=(k == 0), stop=(k == KTILES - 1))
    m_sb = sqp.tile([1, 512], F32R, name="ln_m")
    v_sb = sqp.tile([1, 512], F32R, name="ln_v")
    msq = sqp.tile([1, 512], F32, name="ln_msq")
    nc.vector.tensor_scalar_mul(m_sb[:], mp[:], 1.0 / D)
    nc.vector.tensor_mul(msq[:], m_sb[:], m_sb[:])
    # v = sumsq/D + eps - mean^2
    nc.vector.tensor_scalar(out=v_sb[:], in0=sp[:], scalar1=1.0 / D,
                            scalar2=EPS, op0=ALU.mult, op1=ALU.add)
    nc.vector.tensor_sub(v_sb[:], v_sb[:], msq[:])
    # broadcast mean and var across partitions, then go 128-wide
    mb = bcp.tile([P, 512], F32, name="ln_mb", space="PSUM")
    vb = bcp.tile([P, 512], F32, name="ln_vb", space="PSUM")
    nc.tensor.matmul(mb[:], lhsT=onesr[:], rhs=m_sb[:], start=True, stop=True)
    nc.tensor.matmul(vb[:], lhsT=onesr[:], rhs=v_sb[:], start=True, stop=True)
    rb = xnp.tile([P, 512], F32, name="ln_rb")
    ab = xnp.tile([P, 512], F32, name="ln_ab")
    cb = xnp.tile([P, 512], F32, name="ln_cb")
    nc.vector.reciprocal(rb[:], vb[:])
    nc.scalar.activation(ab[:], rb[:], AF.Sqrt)      # rstd, broadcast
    nc.vector.tensor_mul(cb[:], mb[:], ab[:])        # mean*rstd (subtract later)
    for k in range(KTILES):
        xn = xnp.tile([P, 512], F32, name="ln_xn")
        nc.vector.tensor_mul(xn[:], xT[:, k, pbs], ab[:])
        nc.vector.tensor_sub(xn[:], xn[:], cb[:])
        nc.scalar.activation(xT[:, k, pbs], xn[:], AF.Identity,
                             bias=b_sb[:, k:k + 1], scale=g_sb[:, k:k + 1])


def build():
    nc = bacc.Bacc("TRN2", target_bir_lowering=False, debug=False)

    idx_d = nc.dram_tensor("idx", [P, NT], I32, kind="ExternalInput")
    tok_d = nc.dram_tensor("tok_emb", [VOCAB, D], F32, kind="ExternalInput")
    pos_d = nc.dram_tensor("pos_emb", [L, D], F32, kind="ExternalInput")
    wq_d = nc.dram_tensor("wq", [NH, P, KTILES, KD], F32R, kind="ExternalInput")
    wk_d = nc.dram_tensor("wk", [NH, P, KTILES, KD], F32R, kind="ExternalInput")
    wv_d = nc.dram_tensor("wv", [NH, P, KTILES, KD], F32R, kind="ExternalInput")
    wo_d = nc.dram_tensor("wo", [NH, KD, KTILES, P], F32R, kind="ExternalInput")
    ng_d = nc.dram_tensor("ng", [NH, P, KTILES], F32, kind="ExternalInput")
    nb_d = nc.dram_tensor("nb", [NH, P, KTILES], F32, kind="ExternalInput")
    og_d = nc.dram_tensor("og", [P, KTILES], F32, kind="ExternalInput")
    ob_d = nc.dram_tensor("ob", [P, KTILES], F32, kind="ExternalInput")
    w1_d = nc.dram_tensor("w1", [P, JT, KTILES, P], F32R, kind="ExternalInput")
    b1_d = nc.dram_tensor("b1", [P, JT], F32, kind="ExternalInput")
    w2_d = nc.dram_tensor("w2", [P, NVB, JT, VB], F32R, kind="ExternalInput")
    b2_d = nc.dram_tensor("b2", [NVB, VB], F32R, kind="ExternalInput")
    out_d = nc.dram_tensor("logits", [L, VPC], F32, kind="ExternalOutput")
    xtn_d = nc.dram_tensor("xtn_scratch", [P, KTILES, L], F32R)

    with tile.TileContext(nc) as tc:
        with tc.tile_pool(name="persist", bufs=1) as pp:
            ident = pp.tile([P, P], F32)
            make_identity(nc, ident[:])
            mask = pp.tile([P, P], F32)           # keep k<=q (upper incl diag)
            make_upper_triangular(nc, mask[:], val=1.0, diag=True)
            onesc_f = pp.tile([P, 1], F32)
            onesr_f = pp.tile([1, P], F32)
            nc.vector.memset(onesc_f[:], 1.0)
            nc.vector.memset(onesr_f[:], 1.0)
            onesc = pp.tile([P, 1], F32R)
            onesr = pp.tile([1, P], F32R)
            nc.vector.tensor_copy(onesc[:], onesc_f[:])
            nc.vector.tensor_copy(onesr[:], onesr_f[:])
            idx_sb = pp.tile([P, NT], I32)
            nc.sync.dma_start(idx_sb[:], idx_d[:])
            og_sb = pp.tile([P, KTILES], F32)
            ob_sb = pp.tile([P, KTILES], F32)
            nc.sync.dma_start(og_sb[:], og_d[:])
            nc.sync.dma_start(ob_sb[:], ob_d[:])
            xT = pp.tile([P, KTILES, L], F32R)    # x^T, resident

            # ---------------- embed: gather + pos add + transpose ----------
            with tc.tile_pool(name="emb", bufs=3) as ep, \
                 tc.tile_pool(name="embp", bufs=4, space="PSUM") as epp:
                for t in range(NT):
                    xt = ep.tile([P, D], F32, name="xt")
                    nc.gpsimd.indirect_dma_start(
                        out=xt[:], out_offset=None, in_=tok_d[:],
                        in_offset=IndirectOffsetOnAxis(ap=idx_sb[:, t:t + 1], axis=0),
                    )
                    pe = ep.tile([P, D], F32, name="pe")
                    nc.sync.dma_start(pe[:], pos_d[ts(t, P), :])
                    nc.vector.tensor_add(xt[:], xt[:], pe[:])
                    for k in range(KTILES):
                        tp = epp.tile([P, P], F32, name="tp")
                        nc.tensor.transpose(tp[:], xt[:, ts(k, P)], ident[:])
                        nc.vector.tensor_copy(xT[:, k, ts(t, P)], tp[:])

            # ---------------- attention layers ------------------------------
            for layer in range(NH):
                with tc.tile_pool(name=f"lyr{layer}", bufs=1) as lp:
                    wq_sb = lp.tile([P, KTILES, KD], F32R)
                    wk_sb = lp.tile([P, KTILES, KD], F32R)
                    wv_sb = lp.tile([P, KTILES, KD], F32R)
                    wo_sb = lp.tile([KD, KTILES, P], F32R)
                    ng_sb = lp.tile([P, KTILES], F32)
                    nb_sb = lp.tile([P, KTILES], F32)
                    nc.sync.dma_start(wq_sb[:], wq_d[layer])
                    nc.sync.dma_start(wk_sb[:], wk_d[layer])
                    nc.sync.dma_start(wv_sb[:], wv_d[layer])
                    nc.sync.dma_start(wo_sb[:], wo_d[layer])
                    nc.sync.dma_start(ng_sb[:], ng_d[layer])
                    nc.sync.dma_start(nb_sb[:], nb_d[layer])
                    QT = lp.tile([KD, L], F32R)
                    KT = lp.tile([KD, L], F32R)
                    V_aug = lp.tile([P, NT, KD + 2], F32R)
                    K_pf = lp.tile([P, NT, KD], F32R)
                    nT = lp.tile([KD, L], F32R)
                    S_sb = lp.tile([KD, KD + 2], F32R)
                    zs = lp.tile([KD, KD + 2], F32)
                    zc_f = lp.tile([P, 1], F32)
                    nc.vector.memset(zc_f[:], 0.0)
                    nc.vector.memset(zs[:], 0.0)
                    nc.vector.tensor_copy(S_sb[:], zs[:])

                    # Q^T, K^T projections: [64, 512] psum per pos block
                    with tc.tile_pool(name="projp", bufs=3, space="PSUM") as qp_pool, \
                         tc.tile_pool(name="projs", bufs=3) as qs_pool:
                        for pb in range(NPB):
                            pbs = ts(pb, 512)
                            for w_sb, OUT in ((wq_sb, QT), (wk_sb, KT)):
                                qp = qp_pool.tile([KD, 512], F32, name="qp",
                                                  space="PSUM")
                                for k in range(KTILES):
                                    nc.tensor.matmul(
                                        qp[:], lhsT=w_sb[:, k, :],
                                        rhs=xT[:, k, pbs],
                                        start=(k == 0), stop=(k == KTILES - 1),
                                    )
                                # elu(z)+1 = exp(min(z,0)) + relu(z)
                                t1 = qs_pool.tile([KD, 512], F32, name="t1")
                                t2 = qs_pool.tile([KD, 512], F32, name="t2")
                                nc.vector.tensor_scalar_min(t1[:], qp[:], 0.0)
                                nc.scalar.activation(t1[:], t1[:], AF.Exp)
                                nc.scalar.activation(t2[:], qp[:], AF.Relu)
                                nc.vector.tensor_add(OUT[:, pbs], t1[:], t2[:])

                    # V = x @ Wv in [pos, kd] layout; K_pf = transpose of the
                    # activated KT (S update must see elu(K)+1)
                    with tc.tile_pool(name="vkp", bufs=4, space="PSUM") as vk_pool:
                        for t in range(NT):
                            vp = vk_pool.tile([P, KD], F32, name="vp", space="PSUM")
                            for k in range(KTILES):
                                nc.tensor.matmul(vp[:], lhsT=xT[:, k, ts(t, P)],
                                                 rhs=wv_sb[:, k, :],
                                                 start=(k == 0), stop=(k == KTILES - 1))
                            nc.vector.tensor_copy(V_aug[:, t, 0:KD], vp[:])
                            nc.vector.tensor_copy(V_aug[:, t, KD:KD + 1], onesc_f[:])
                            nc.vector.tensor_copy(V_aug[:, t, KD + 1:KD + 2],
                                                  zc_f[:])
                            ktr = vk_pool.tile([P, KD], F32, name="ktr", space="PSUM")
                            nc.tensor.transpose(
                                ktr[:], KT[:, ts(t, P)].bitcast(F32),
                                ident[:KD, :KD])
                            nc.vector.tensor_copy(K_pf[:, t, :], ktr[:])

                    # causal chunk scan
                    with tc.tile_pool(name="scanp", bufs=2, space="PSUM") as sc_pool, \
                         tc.tile_pool(name="sps", bufs=1, space="PSUM") as s_pool, \
                         tc.tile_pool(name="scans", bufs=3) as ss_pool:
                        S_ps = s_pool.tile([KD, KD + 2], F32, name="s_ps",
                                           space="PSUM")
                        for t in range(NT):
                            tts = ts(t, P)
                            at = sc_pool.tile([P, P], F32, name="at", space="PSUM")
                            nc.tensor.matmul(at[:], lhsT=KT[:, tts], rhs=QT[:, tts],
                                             start=True, stop=True)
                            atm = ss_pool.tile([P, P], F32R, name="atm")
                            nc.vector.tensor_tensor(
                                out=atm[:], in0=at[:], in1=mask[:], op=ALU.mult)
                            np_ = sc_pool.tile([P, KD + 2], F32, name="np",
                                               space="PSUM")
                            nc.tensor.matmul(np_[:], lhsT=atm[:], rhs=V_aug[:, t, :],
                                             start=True, stop=False)
                            nc.tensor.matmul(np_[:], lhsT=QT[:, tts], rhs=S_sb[:],
                                             start=False, stop=True)
                            nc.tensor.matmul(S_ps[:], lhsT=K_pf[:, t, :],
                                             rhs=V_aug[:, t, :],
                                             start=(t == 0), stop=(t == NT - 1))
                            nc.vector.tensor_copy(S_sb[:], S_ps[:])
                            rd = ss_pool.tile([P, 1], F32, name="rd")
                            rec = ss_pool.tile([P, 1], F32, name="rec")
                            n_sb = ss_pool.tile([P, KD], F32, name="n_sb")
                            nc.vector.tensor_scalar_add(rd[:], np_[:, KD:KD + 1],
                                                        DEN_EPS)
                            nc.vector.reciprocal(rec[:], rd[:])
                            nc.vector.tensor_scalar_mul(n_sb[:], np_[:, 0:KD], rec[:])
                            tr = sc_pool.tile([KD, P], F32, name="tr", space="PSUM")
                            nc.tensor.transpose(tr[:], n_sb[:], ident[:])
                            nc.vector.tensor_copy(nT[:, tts], tr[:])

                    # Wo apply + residual + LN, per pos block
                    with tc.tile_pool(name="wop", bufs=2, space="PSUM") as wo_pool, \
                         tc.tile_pool(name="lnp", bufs=1, space="PSUM") as ln_pool, \
                         tc.tile_pool(name="lnsq", bufs=2) as sq_pool, \
                         tc.tile_pool(name="lnxn", bufs=2) as xn_pool, \
                         tc.tile_pool(name="lnbc", bufs=1, space="PSUM") as bc_pool:
                        for pb in range(NPB):
                            pbs = ts(pb, 512)
                            for k in range(KTILES):
                                yp = wo_pool.tile([P, 512], F32, name="yp",
                                                  space="PSUM")
                                nc.tensor.matmul(yp[:], lhsT=wo_sb[:, k, :],
                                                 rhs=nT[:, pbs],
                                                 start=True, stop=True)
                                nc.vector.tensor_add(xT[:, k, pbs], xT[:, k, pbs],
                                                     yp[:])
                            _layernorm_block(
                                nc, (ln_pool, sq_pool, xn_pool, bc_pool),
                                xT, pb, ng_sb, nb_sb, onesc, onesr)

            # ---------------- final LN + spill xT ---------------------------
            with tc.tile_pool(name="flnp", bufs=1, space="PSUM") as ln_pool, \
                 tc.tile_pool(name="flnsq", bufs=2) as sq_pool, \
                 tc.tile_pool(name="flnxn", bufs=2) as xn_pool, \
                 tc.tile_pool(name="flnbc", bufs=1, space="PSUM") as bc_pool:
                for pb in range(NPB):
                    _layernorm_block(
                        nc, (ln_pool, sq_pool, xn_pool, bc_pool),
                        xT, pb, og_sb, ob_sb, onesc, onesr)
                    # spill this block right away so the MLP can start on
                    # half 0 while half 1 is still normalizing
                    for k in range(KTILES):
                        nc.sync.dma_start(xtn_d[:, k, ts(pb, 512)],
                                          xT[:, k, ts(pb, 512)])

        # ---------------- MLP ----------------------------------------------
        with tc.tile_pool(name="mlp", bufs=1) as mp_pool:
            b1_sb = mp_pool.tile([P, JT], F32)
            nc.sync.dma_start(b1_sb[:], b1_d[:])
            onesr2_f = mp_pool.tile([1, P], F32)
            nc.vector.memset(onesr2_f[:], 1.0)
            onesr2 = mp_pool.tile([1, P], F32R)
            nc.vector.tensor_copy(onesr2[:], onesr2_f[:])
            hT = mp_pool.tile([P, JT, HALF], F32R)
            for half in range(2):
                hoff = half * HALF
                # ---- W1 GEMM + gelu -> hT ----
                with tc.tile_pool(name="w1x", bufs=2) as xp, \
                     tc.tile_pool(name="w1s", bufs=3) as w1s, \
                     tc.tile_pool(name="w1p", bufs=3, space="PSUM") as hp1:
                    xhbs = []
                    for hb in range(2):
                        xhb = xp.tile([P, KTILES, 512], F32R, name="xhb")
                        nc.sync.dma_start(
                            xhb[:], xtn_d[:, :, hoff + hb * 512:hoff + (hb + 1) * 512])
                        xhbs.append(xhb)
                    for j in range(JT):
                        w1t = w1s.tile([P, KTILES, P], F32R, name="w1t")
                        nc.sync.dma_start(w1t[:], w1_d[:, j, :, :])
                        for hb in range(2):
                            ps = hp1.tile([P, 512], F32, name="hps", space="PSUM")
                            for k in range(KTILES):
                                nc.tensor.matmul(ps[:], lhsT=w1t[:, k, :],
                                                 rhs=xhbs[hb][:, k, :],
                                                 start=(k == 0),
                                                 stop=(k == KTILES - 1))
                            nc.scalar.activation(
                                hT[:, j, ts(hb, 512)], ps[:], AF.Gelu,
                                bias=b1_sb[:, j:j + 1], scale=1.0)
                # ---- W2 GEMM -> logits ----
                with tc.tile_pool(name="w2s", bufs=2) as w2p, \
                     tc.tile_pool(name="w2lg", bufs=3) as lgp, \
                     tc.tile_pool(name="w2b", bufs=2) as b2p, \
                     tc.tile_pool(name="w2ps", bufs=4, space="PSUM") as lp2, \
                     tc.tile_pool(name="w2bp", bufs=1, space="PSUM") as bp2:
                    for vb in range(NVB):
                        w2t = w2p.tile([P, JT, VB], F32R, name="w2t")
                        nc.sync.dma_start(w2t[:], w2_d[:, vb, :, :])
                        b2v = b2p.tile([1, VB], F32R, name="b2v")
                        nc.sync.dma_start(b2v[:], b2_d[vb:vb + 1, :])
                        b2ps = bp2.tile([P, VB], F32, name="b2ps", space="PSUM")
                        nc.tensor.matmul(b2ps[:], lhsT=onesr2[:], rhs=b2v[:],
                                         start=True, stop=True)
                        b2b = b2p.tile([P, VB], F32, name="b2b")
                        nc.vector.tensor_copy(b2b[:], b2ps[:])
                        for pt in range(8):
                            ps = lp2.tile([P, VB], F32, name="lps", space="PSUM")
                            for k in range(JT):
                                nc.tensor.matmul(ps[:], lhsT=hT[:, k, ts(pt, P)],
                                                 rhs=w2t[:, k, :],
                                                 start=(k == 0), stop=(k == JT - 1))
                            lg = lgp.tile([P, VB], F32, name="lg")
                            nc.vector.tensor_add(lg[:], ps[:], b2b[:])
                            nc.sync.dma_start(
                                out_d[hoff + pt * P:hoff + (pt + 1) * P,
                                      ts(vb, VB)], lg[:])
    nc.compile()
    return nc


def _prep_core_inputs(inputs, batch, vslice):
    f32 = np.float32
    idx = np.asarray(inputs["input_indices"])[batch].astype(np.int32)
    idxr = np.ascontiguousarray(idx.reshape(NT, P).T)
    tok = np.ascontiguousarray(np.asarray(inputs["token_embed"], dtype=f32))
    pos = np.ascontiguousarray(np.asarray(inputs["pos_embed"], dtype=f32)[:L])
    wq = np.asarray(inputs["Wq"], dtype=f32).reshape(NH, KTILES, P, KD)
    wk = np.asarray(inputs["Wk"], dtype=f32).reshape(NH, KTILES, P, KD)
    wv = np.asarray(inputs["Wv"], dtype=f32).reshape(NH, KTILES, P, KD)
    wqr = np.ascontiguousarray(wq.transpose(0, 2, 1, 3))
    wkr = np.ascontiguousarray(wk.transpose(0, 2, 1, 3))
    wvr = np.ascontiguousarray(wv.transpose(0, 2, 1, 3))
    wo = np.ascontiguousarray(
        np.asarray(inputs["Wo"], dtype=f32).reshape(NH, KD, KTILES, P))
    ng = np.ascontiguousarray(
        np.asarray(inputs["norm_g"], dtype=f32).reshape(NH, KTILES, P)
        .transpose(0, 2, 1))
    nb = np.ascontiguousarray(
        np.asarray(inputs["norm_b"], dtype=f32).reshape(NH, KTILES, P)
        .transpose(0, 2, 1))
    og = np.ascontiguousarray(
        np.asarray(inputs["out_norm_g"], dtype=f32).reshape(KTILES, P).T)
    ob = np.ascontiguousarray(
        np.asarray(inputs["out_norm_b"], dtype=f32).reshape(KTILES, P).T)
    w1 = np.asarray(inputs["W1"], dtype=f32).reshape(KTILES, P, JT, P)
    w1r = np.ascontiguousarray(w1.transpose(1, 2, 0, 3))
    b1r = np.ascontiguousarray(
        np.asarray(inputs["b1"], dtype=f32).reshape(JT, P).T)
    w2 = np.asarray(inputs["W2"], dtype=f32)
    vs = VOCAB // 4
    w2p = np.zeros((HID, VPC), dtype=f32)
    w2p[:, :vs] = w2[:, vslice * vs:(vslice + 1) * vs]
    w2r = np.ascontiguousarray(
        w2p.reshape(JT, P, NVB, VB).transpose(1, 2, 0, 3))
    b2 = np.asarray(inputs["b2"], dtype=f32)
    b2p = np.zeros((VPC,), dtype=f32)
    b2p[:vs] = b2[vslice * vs:(vslice + 1) * vs]
    b2r = b2p.reshape(NVB, VB)
    return {
        "idx": idxr, "tok_emb": tok, "pos_emb": pos,
        "wq": wqr, "wk": wkr, "wv": wvr, "wo": wo,
        "ng": ng, "nb": nb, "og": og, "ob": ob,
        "w1": w1r, "b1": b1r, "w2": w2r, "b2": b2r,
    }


def kernel(**inputs) -> np.ndarray:
    global _cached
    if _cached is None:
        _cached = build()
    nc = _cached
    in_maps = [_prep_core_inputs(inputs, c // 4, c % 4) for c in range(8)]
    r = run_bass_kernel_spmd(nc, in_maps, core_ids=list(range(8)))
    vs = VOCAB // 4
    B = np.asarray(inputs["input_indices"]).shape[0]
    out = np.empty((B, L, VOCAB), dtype=np.float32)
    for c in range(8):
        b, v = c // 4, c % 4
        out[b, :, v * vs:(v + 1) * vs] = r.results[c]["logits"][:, :vs]
    return out
